# revision 5
# baseline (speedup 1.0000x reference)
"""Trainium2 Bass kernel for nn_ContactMapDistError — v3.

v2 split the PSUM drain across DVE (direct tensor_reduce) and ACT
(fp32->fp16 copy + DVE fp16 min-tournament at 2x). v3 fixes the
pipeline coupling that kept v2 at ~97us despite ~62us engine busy:

  - ONE [128, 4096] PSUM tensor = FOUR 768-col chunk buffers (1024
    stride), so up to 3 consumers run behind the PE instead of 1
  - ACT copies whole runs of consecutive A-chunks (up to 3) in a
    single strided instruction: one ~217ns startup per 3 chunks
  - chunk = 768 w = 8 s-groups; 108 chunks; D:A = 28:80
  - tournament pieces (L2..TR) drip-flushed between D-reduces so no
    DVE burst delays a PSUM-freeing reduce
"""

import sys

sys.path.insert(0, "/opt/trn_rl_repo")

import numpy as np

import concourse.bass as bass
import concourse.mybir as mybir
from concourse.bass_utils import run_bass_kernel_spmd

F32 = mybir.dt.float32
F32R = mybir.dt.float32r
F16 = mybir.dt.float16

B, N, R, VR = 4, 10475, 48, 96
NCORES = 8
RH = R // 2            # r-regions per core
V = RH * VR            # packed v columns per core = 2304
T = V // 128           # v-chunks of 128 partitions = 18
W = R * VR             # full w width = 4608
WC = 768               # w-chunk (8 s-groups); lives in a 1024-col psum slot
NWC = W // WC          # = 6
K = 5                  # contraction dim
NK = T * NWC           # total chunk count = 108
SG = WC // VR          # s-groups per chunk = 8
NPT = 4                # psum chunk buffers

GRP = 8                # A-chunks per tournament group
ROUTE = ["A"] * NK     # 28 D, 80 A
for _k in range(NK):
    if _k % 4 == 0:
        ROUTE[_k] = "D"
ROUTE[106] = "D"
D_RANK, A_RANK = {}, {}
for _k in range(NK):
    if ROUTE[_k] == "D":
        D_RANK[_k] = len(D_RANK)
    else:
        A_RANK[_k] = len(A_RANK)
ND, NA = len(D_RANK), len(A_RANK)
assert NA % GRP == 0
NGRP = NA // GRP       # 10
ACOL0 = SG * ND        # first s1buf column of route-A outputs = 224
LHALF = V // 2

# consecutive-k runs of A-chunks (each run becomes one ACT copy instr)
A_RUNS = []
_run = []
for _k in range(NK):
    if ROUTE[_k] == "A":
        if _run and (A_RANK[_k] % (2 * GRP) == 0 or _k % NPT == 0):
            A_RUNS.append(_run)   # scratch or psum ring wrapped: new run
            _run = []
        _run.append(_k)
        if len(_run) == 2:
            A_RUNS.append(_run)
            _run = []
    else:
        if _run:
            A_RUNS.append(_run)
            _run = []
if _run:
    A_RUNS.append(_run)
# act_sem value observed once the run-copy containing A-rank j completes
ACT_GATE = [0] * NA
_post = 0
for _r in A_RUNS:
    _post += len(_r)
    for _kk in _r:
        ACT_GATE[A_RANK[_kk]] = _post

_cache = {}


def _build():
    if "nc" in _cache:
        return _cache["nc"]
    nc = bass.Bass()
    ab = nc.declare_dram_parameter("ab", [K, V + W], F32R, isOutput=False)
    s1out = nc.declare_dram_parameter("s1out", [128, T * R], F32, isOutput=True)

    abt = nc.alloc_sbuf_tensor("abt", [K, V + W], F32R).ap()
    s1buf = nc.alloc_sbuf_tensor("s1buf", [128, T * R], F32).ap()
    scr = nc.alloc_sbuf_tensor("scr", [128, 2 * GRP, SG, VR], F16).ap()
    l1w_ = nc.alloc_sbuf_tensor("l1w", [128, 2, GRP, SG, 48], F16).ap()
    l2w_ = nc.alloc_sbuf_tensor("l2w", [128, 2, GRP, SG, 24], F16).ap()
    l3w_ = nc.alloc_sbuf_tensor("l3w", [128, 2, GRP, SG, 12], F16).ap()
    l4w_ = nc.alloc_sbuf_tensor("l4w", [128, 2, GRP, SG, 6], F16).ap()
    PT = nc.alloc_psum_tensor("PT", [128, 4096], F32).ap()

    def pt(k):
        return PT[:, 1024 * (k % NPT) : 1024 * (k % NPT) + WC]

    lt = abt[:, 0:V]
    rt = abt[:, V : V + W]

    # statically compute the tsem count at which each group's TR completes,
    # mirroring the vector-engine emission order below
    tr_count = [0] * NGRP
    _tc = 0
    _pend = []  # group index (for TR) or None per pending piece
    for _k in range(NK):
        if ROUTE[_k] == "D":
            if _pend:
                _g0 = _pend.pop(0)
                _tc += 1
                if _g0 is not None:
                    tr_count[_g0] = _tc
            continue
        _j = A_RANK[_k]
        if _j % 4 == 3:
            _tc += 1  # inline L1 piece
            if _pend:
                _g0 = _pend.pop(0)
                _tc += 1
                if _g0 is not None:
                    tr_count[_g0] = _tc
        if _j % GRP == GRP - 1:
            _pend.extend([None, None, None, _j // GRP])
    for _g0 in _pend:
        _tc += 1
        if _g0 is not None:
            tr_count[_g0] = _tc

    with (
        nc.Block() as block,
        nc.semaphore("dl0") as dl0,
        nc.semaphore("dl1") as dl1,
        nc.semaphore("dr0") as dr0,
        nc.semaphore("dr1") as dr1,
        nc.semaphore("dr2") as dr2,
        nc.semaphore("odma") as odma,
        nc.semaphore("pe_sem") as pe_sem,
        nc.semaphore("dsem") as dsem,
        nc.semaphore("act_sem") as act_sem,
        nc.semaphore("tsem") as tsem,
    ):

        @block.sync
        def _(sp):
            sp.dma_start(abt[:, 0:LHALF], ab[:, 0:LHALF]).then_inc(dl0, 16)
            # stream results out (HWDGE queue) as tournament group PAIRS finish
            for gi in range(1, NGRP, 2):
                sp.wait_ge(tsem, tr_count[gi])
                sp.dma_start(
                    s1out[:, ACOL0 + 64 * (gi - 1) : ACOL0 + 64 * (gi + 1)],
                    s1buf[:, ACOL0 + 64 * (gi - 1) : ACOL0 + 64 * (gi + 1)],
                ).then_inc(odma, 16)
            sp.wait_ge(dsem, ND)
            sp.dma_start(s1out[:, 0:ACOL0], s1buf[:, 0:ACOL0]).then_inc(odma, 16)

        @block.gpsimd
        def _(g):
            g.dma_start(abt[:, V : V + 1536], ab[:, V : V + 1536]).then_inc(dr0, 16)
            g.dma_start(
                abt[:, V + 3072 : V + 4608], ab[:, V + 3072 : V + 4608]
            ).then_inc(dr2, 16)
            g.wait_ge(odma, 16 * (NGRP // 2 + 1))

        @block.tensor
        def _(pe):
            pe.wait_ge(dl0, 16)
            pe.wait_ge(dr0, 16)
            for k in range(NK):
                t, c = divmod(k, NWC)
                if k == 2:
                    pe.wait_ge(dr1, 16)
                elif k == 4:
                    pe.wait_ge(dr2, 16)
                elif t == T // 2 and c == 0:
                    pe.wait_ge(dl1, 16)
                if k >= NPT:
                    # wait for the consumer of chunk k-NPT (same psum slot)
                    p = k - NPT
                    if ROUTE[p] == "D":
                        pe.wait_ge(dsem, D_RANK[p] + 1)
                    else:
                        pe.wait_ge(act_sem, ACT_GATE[A_RANK[p]])
                dst = pt(k)
                pe.matmul(
                    dst[:, 0:512],
                    lt[:, t * 128 : (t + 1) * 128],
                    rt[:, c * WC : c * WC + 512],
                    start=True,
                    stop=True,
                )
                pe.matmul(
                    dst[:, 512:768],
                    lt[:, t * 128 : (t + 1) * 128],
                    rt[:, c * WC + 512 : c * WC + 768],
                    start=True,
                    stop=True,
                ).then_inc(pe_sem)

        @block.scalar
        def _(act):
            act.dma_start(
                abt[:, V + 1536 : V + 3072], ab[:, V + 1536 : V + 3072]
            ).then_inc(dr1, 16)
            act.dma_start(abt[:, LHALF:V], ab[:, LHALF:V]).then_inc(dl1, 16)
            for run in A_RUNS:
                j0 = A_RANK[run[0]]
                # scratch-slot reuse gate (slots of parity (g%2) were last
                # read by group g-2's L1 pieces, all before TR(g-2))
                for kk in run:
                    jj = A_RANK[kk]
                    gg = jj // GRP
                    if jj % GRP == 0 and gg >= 2:
                        act.wait_ge(tsem, tr_count[gg - 2])
                act.wait_ge(pe_sem, run[-1] + 1)
                slot0 = j0 % (2 * GRP)
                nrun = len(run)
                # psum slots of a run are consecutive (k consecutive, mod 4
                # never wraps: runs start at k%4==1) and scr slots are
                # consecutive (j consecutive, runs don't cross the 16-ring)
                src = PT[:, 1024 * (run[0] % NPT) :].rearrange(
                    "p (n q) -> p n q", q=1024
                )[:, 0:nrun, 0:WC]
                dst = scr[:, slot0 : slot0 + nrun]
                act.activation(
                    dst.rearrange("p a b c -> p a (b c)"),
                    src,
                    mybir.ActivationFunctionType.Copy,
                ).then_inc(act_sem, nrun)

        @block.vector
        def _(v):
            mn = mybir.AluOpType.min
            tcnt = 0
            pend = []

            def emit(fn):
                nonlocal tcnt
                v.wait_ge(tsem, tcnt)
                fn().then_inc(tsem)
                tcnt += 1

            def flush(n=1):
                for _ in range(min(n, len(pend))):
                    emit(pend.pop(0))

            for k in range(NK):
                if ROUTE[k] == "D":
                    d = D_RANK[k]
                    v.wait_ge(pe_sem, k + 1)
                    v.tensor_reduce(
                        s1buf[:, SG * d : SG * (d + 1)],
                        pt(k).rearrange("p (g v) -> p g v", v=VR),
                        axis=mybir.AxisListType.X,
                        op=mn,
                    ).then_inc(dsem)
                    flush(1)
                    continue
                j = A_RANK[k]
                g = j // GRP
                s0 = (g % 2) * GRP
                l1w = l1w_[:, g % 2]
                l2w = l2w_[:, g % 2]
                l3w = l3w_[:, g % 2]
                l4w = l4w_[:, g % 2]
                if j % 4 == 3:
                    # L1 piece over the 4 chunks just copied
                    p4 = (j % GRP) // 4
                    v.wait_ge(act_sem, ACT_GATE[j])
                    v.tensor_tensor(
                        l1w[:, 4 * p4 : 4 * p4 + 4],
                        scr[:, s0 + 4 * p4 : s0 + 4 * p4 + 4, :, 0:48],
                        scr[:, s0 + 4 * p4 : s0 + 4 * p4 + 4, :, 48:96],
                        op=mn,
                    ).then_inc(tsem)
                    tcnt += 1
                    flush(1)
                if j % GRP == GRP - 1:
                    pend.append(lambda l1w=l1w, l2w=l2w: v.tensor_tensor(
                        l2w, l1w[:, :, :, 0:24], l1w[:, :, :, 24:48], op=mn))
                    pend.append(lambda l2w=l2w, l3w=l3w: v.tensor_tensor(
                        l3w, l2w[:, :, :, 0:12], l2w[:, :, :, 12:24], op=mn))
                    pend.append(lambda l3w=l3w, l4w=l4w: v.tensor_tensor(
                        l4w, l3w[:, :, :, 0:6], l3w[:, :, :, 6:12], op=mn))

                    def tr_fn(g=g, l4w=l4w):
                        r = v.tensor_reduce(
                            s1buf[:, ACOL0 + 64 * g : ACOL0 + 64 * (g + 1)],
                            l4w.rearrange("p a b c -> p (a b) c"),
                            axis=mybir.AxisListType.X,
                            op=mn,
                        )
                        assert tr_count[g] == tcnt + 1, (g, tr_count[g], tcnt)
                        return r

                    pend.append(tr_fn)
            flush(len(pend))
            assert tcnt == _tc, (tcnt, _tc)

    _cache["nc"] = nc
    return nc


def _prep_inputs(v1s, v2s, rid_to_vid):
    """Build per-core fused lhsT|rhs feature matrices (as baseline)."""
    g1 = v1s[:, rid_to_vid, :]  # [B, R, VR, 3]
    g2 = v2s[:, rid_to_vid, :]
    g1_64 = g1.astype(np.float64)
    g2_64 = g2.astype(np.float64)
    sq1 = (g1_64 * g1_64).sum(-1)
    sq2 = (g2_64 * g2_64).sum(-1)

    in_maps = []
    for core in range(NCORES):
        b, h = divmod(core, 2)
        rs = slice(RH * h, RH * (h + 1))
        a = np.empty((K, V + W), np.float32)
        a[0:3, 0:V] = -2.0 * g1[b, rs].reshape(V, 3).T
        a[3, 0:V] = sq1[b, rs].reshape(V).astype(np.float32)
        a[4, 0:V] = 1.0
        a[0:3, V:] = g2[b].reshape(W, 3).T
        a[3, V:] = 1.0
        a[4, V:] = sq2[b].reshape(W).astype(np.float32)
        in_maps.append({"ab": a})
    return in_maps


# chunk k -> first s1buf column of its SG s-group minima
_COL0 = np.array(
    [SG * D_RANK[k] if ROUTE[k] == "D" else ACOL0 + SG * A_RANK[k]
     for k in range(NK)],
    dtype=np.int64,
)


def _assemble(res):
    md2 = np.empty((B, R, R), np.float32)
    for core in range(NCORES):
        b, h = divmod(core, 2)
        out = res.results[core]["s1out"]  # [128, T*R]
        per_v = np.empty((V, R), np.float32)
        for k in range(NK):
            t, c = divmod(k, NWC)
            per_v[t * 128 : (t + 1) * 128, c * SG : (c + 1) * SG] = (
                out[:, _COL0[k] : _COL0[k] + SG]
            )
        md2[b, RH * h : RH * (h + 1), :] = per_v.reshape(RH, VR, R).min(axis=1)
    return md2


def kernel(v1s, v2s, cmaps, rid_to_vid):
    v1s = np.asarray(v1s)
    v2s = np.asarray(v2s)
    cmaps = np.asarray(cmaps)
    rid_to_vid = np.asarray(rid_to_vid)

    nc = _build()
    in_maps = _prep_inputs(v1s, v2s, rid_to_vid)
    res = run_bass_kernel_spmd(nc, in_maps, core_ids=list(range(NCORES)))
    md2 = _assemble(res)

    md = np.sqrt(np.maximum(md2, 0.0))
    m = cmaps.astype(np.float32)
    return ((md * m).sum(axis=(1, 2)) / m.sum(axis=(1, 2))).astype(np.float32)
